# revision 1
# baseline (speedup 1.0000x reference)
"""DISCO downsample conv (3x3, stride 2, pad 1) on 8 Trainium2 NeuronCores.

Strategy:
  - Effective weights w[o,i,kh,kw] = sum_b coeff[o,i,b]*basis[b,kh,kw] are tiny:
    computed on host, shipped per-tap transposed as wt[i, tap, o] (bf16).
  - x is zero-padded (H+2, W+2) on host, W phase-split into [even | odd]
    columns so every conv tap reads a contiguous run of 256 columns, and cast
    to bf16 (halves HBM traffic; PSUM accumulation stays fp32).
  - Sharding: 8 shards = (batch b in 0..3) x (H half in 0..1). Each core gets
    padded rows [256*h, 256*h + 257) of batch b -- the 1-row halo is part of
    the shard, so no inter-core communication is needed.
  - Per core: stream 8 row-blocks (16 output rows each = 33 input rows).
    For each pair of output rows: one PSUM bank [96, 512], accumulate 9
    matmuls (K=96 in-channels, M=96 out-channels, N=512 pixels), then
    ScalarE adds bias while copying PSUM -> SBUF, and the block is DMA'd out.
"""

import os
import sys
import types

import numpy as np


# ----------------------------------------------------------------------------
# Environment bootstrap (self-contained: no reads from /root/problem).
# ----------------------------------------------------------------------------
def _ensure_paths():
    for p in (
        "/root/.axon_site",
        "/root/.axon_site/_ro/trn_rl_repo",
        "/root/.axon_site/_ro/pypackages",
        "/opt/trn_rl_repo",
    ):
        if os.path.isdir(p) and p not in sys.path:
            sys.path.append(p)


_ensure_paths()

import ml_dtypes  # noqa: E402


def _install_ntff_hook():
    """Register the NTFF profile hook (used when tracing; harmless otherwise)."""
    try:
        import antenv
    except ImportError:
        return
    if "antenv.axon_hooks" not in sys.modules:
        hooks_mod = types.ModuleType("antenv.axon_hooks")
        _hook = [None]
        hooks_mod.set_axon_ntff_profile_hook = lambda h: _hook.__setitem__(0, h)
        hooks_mod.get_axon_ntff_profile_hook = lambda: _hook[0]
        sys.modules["antenv.axon_hooks"] = hooks_mod
        antenv.axon_hooks = hooks_mod
    from antenv.axon_hooks import (
        get_axon_ntff_profile_hook,
        set_axon_ntff_profile_hook,
    )

    if get_axon_ntff_profile_hook() is None:
        try:
            from trn_agent_boot.trn_boot import _ntff_profile_via_ctypes

            so = "/opt/axon/libaxon_pjrt.so"
            if os.path.exists(so):
                set_axon_ntff_profile_hook(_ntff_profile_via_ctypes(so))
        except Exception:
            pass


_install_ntff_hook()

import concourse.bass as bass  # noqa: E402
import concourse.tile as tile  # noqa: E402
from concourse import bacc, mybir  # noqa: E402
import concourse.bass_utils as _bu  # noqa: E402

# Artifact upload needs a bucket that isn't reachable here; keep traces local.
_bu.upload_artifacts = lambda tmpdir: f"local:{tmpdir}"

BF16 = mybir.dt.bfloat16
F32 = mybir.dt.float32
NP_BF16 = ml_dtypes.bfloat16

C = 96          # channels (in == out)
B = 8           # num basis
K = 3           # kernel size
N_CORES = 8
H = W = 512     # input spatial
HO = WO = 256   # output spatial
HP = H + 2      # padded rows
WP = W + 2      # padded cols (phase-split: [257 even | 257 odd])
SH_ROWS = 257   # padded rows per shard (256 + 1 halo)
CORE_HO = 128   # output rows per core
BH = 16         # output rows per block
NBLK = CORE_HO // BH
IN_ROWS = 2 * BH + 1  # input rows per block (33)

# column base per kw tap: even-phase col 2*ow -> slot ow (base 0);
# odd-phase col 2*ow+1 -> slot 257+ow; even col 2*ow+2 -> slot ow+1.
_KW_BASE = {0: 0, 1: 257, 2: 1}

_PROGRAM_CACHE = {}


def _build_program():
    """One SPMD Bass program, shared by all 8 cores."""
    nc = bacc.Bacc()
    x_d = nc.dram_tensor("x", [C, SH_ROWS, WP], BF16, kind="ExternalInput")
    w_d = nc.dram_tensor("wt", [C, K * K, C], BF16, kind="ExternalInput")
    b_d = nc.dram_tensor("bias", [C, 1], F32, kind="ExternalInput")
    y_d = nc.dram_tensor("out", [C, CORE_HO, WO], F32, kind="ExternalOutput")

    with tile.TileContext(nc) as tc:
        with (
            tc.tile_pool(name="const", bufs=1) as cpool,
            tc.tile_pool(name="xin", bufs=3) as xpool,
            tc.tile_pool(name="oout", bufs=2) as opool,
            tc.tile_pool(name="psum", bufs=8, space=bass.MemorySpace.PSUM) as ppool,
        ):
            wt = cpool.tile([C, K * K, C], BF16)
            nc.sync.dma_start(wt[:], w_d[:])
            bias = cpool.tile([C, 1], F32)
            nc.sync.dma_start(bias[:], b_d[:])

            for blk in range(NBLK):
                xt = xpool.tile([C, IN_ROWS, WP], BF16)
                nc.sync.dma_start(
                    xt[:], x_d[:, 2 * BH * blk : 2 * BH * blk + IN_ROWS, :]
                )
                out_sb = opool.tile([C, BH, WO], F32)
                for t in range(BH // 2):
                    ps = ppool.tile([C, 2 * WO], F32)
                    for tap in range(K * K):
                        kh, kw = tap // K, tap % K
                        cb = _KW_BASE[kw]
                        rhs = xt[:, 4 * t + kh : 4 * t + kh + 3 : 2, cb : cb + WO]
                        nc.tensor.matmul(
                            ps[:],
                            wt[:, tap, :],
                            rhs,
                            start=(tap == 0),
                            stop=(tap == K * K - 1),
                        )
                    nc.scalar.activation(
                        out_sb[:, 2 * t : 2 * t + 2, :],
                        ps[:].rearrange("p (a b) -> p a b", a=2),
                        mybir.ActivationFunctionType.Identity,
                        bias=bias[:],
                    )
                nc.sync.dma_start(
                    y_d[:, BH * blk : BH * blk + BH, :], out_sb[:]
                )

    nc.compile()
    return nc


def _get_program():
    if "nc" not in _PROGRAM_CACHE:
        _PROGRAM_CACHE["nc"] = _build_program()
    return _PROGRAM_CACHE["nc"]


def _prepare_inputs(x, coeff, basis, bias):
    """Host prep: effective weights, padded phase-split bf16 x, shards."""
    # wt[i, tap, o] = sum_b coeff[o,i,b] * basis[b, tap]
    w_eff = (
        coeff.astype(np.float32).reshape(C * C, B)
        @ basis.astype(np.float32).reshape(B, K * K)
    ).reshape(C, C, K * K)
    wt = np.ascontiguousarray(w_eff.transpose(1, 2, 0)).astype(NP_BF16)

    xb = x.astype(NP_BF16)
    xph = np.zeros((x.shape[0], C, HP, WP), dtype=NP_BF16)
    # even phase: padded col 2j -> orig col 2j-1  (slot j=1..256)
    xph[:, :, 1 : H + 1, 1:257] = xb[:, :, :, 1::2]
    # odd phase: padded col 2j+1 -> orig col 2j  (slot 257+j, j=0..255)
    xph[:, :, 1 : H + 1, 257:513] = xb[:, :, :, 0::2]

    bias2 = np.ascontiguousarray(bias.astype(np.float32).reshape(C, 1))

    in_maps = []
    for s in range(N_CORES):
        b_idx, h_idx = divmod(s, 2)
        shard = np.ascontiguousarray(
            xph[b_idx, :, 256 * h_idx : 256 * h_idx + SH_ROWS, :]
        )
        in_maps.append({"x": shard, "wt": wt, "bias": bias2})
    return in_maps


def _assemble(results, n_batch):
    out = np.empty((n_batch, C, 2 * CORE_HO, WO), dtype=np.float32)
    for s in range(N_CORES):
        b_idx, h_idx = divmod(s, 2)
        out[b_idx, :, CORE_HO * h_idx : CORE_HO * (h_idx + 1), :] = results[s]["out"]
    return out


def run(x, coeff, basis, bias, trace=False, trace_cores=None):
    """Run the kernel; returns (full_output, BassKernelResults)."""
    nc = _get_program()
    in_maps = _prepare_inputs(x, coeff, basis, bias)
    res = _bu.run_bass_kernel_spmd(
        nc,
        in_maps,
        list(range(N_CORES)),
        trace=trace,
        trace_cores=trace_cores,
    )
    return _assemble(res.results, x.shape[0]), res


def kernel(x, coeff, basis, bias):
    out, _ = run(x, coeff, basis, bias, trace=False)
    return out


# revision 3
# speedup vs baseline: 1.1352x; 1.1352x over previous
"""DISCO downsample conv (3x3, stride 2, pad 1) on 8 Trainium2 NeuronCores.

Strategy:
  - Effective weights w[o,i,kh,kw] = sum_b coeff[o,i,b]*basis[b,kh,kw] are tiny:
    computed on host, shipped per-tap transposed as wt[i, tap, o] (bf16).
  - x is zero-padded (H+2, W+2) on host, W phase-split into [even | odd]
    columns so every conv tap reads a contiguous run of 256 columns, and cast
    to bf16 (halves HBM traffic; PSUM accumulation stays fp32).
  - Sharding: 8 shards = (batch b in 0..3) x (H half in 0..1). Each core gets
    padded rows [256*h, 256*h + 257) of batch b -- the 1-row halo is part of
    the shard, so no inter-core communication is needed.
  - Per core: stream 8 row-blocks (16 output rows each = 33 input rows).
    For each pair of output rows: one PSUM bank [96, 512], accumulate 9
    matmuls (K=96 in-channels, M=96 out-channels, N=512 pixels), then
    ScalarE adds bias while copying PSUM -> SBUF, and the block is DMA'd out.
"""

import os
import sys
import types

import numpy as np


# ----------------------------------------------------------------------------
# Environment bootstrap (self-contained: no reads from /root/problem).
# ----------------------------------------------------------------------------
def _ensure_paths():
    for p in (
        "/root/.axon_site",
        "/root/.axon_site/_ro/trn_rl_repo",
        "/root/.axon_site/_ro/pypackages",
        "/opt/trn_rl_repo",
    ):
        if os.path.isdir(p) and p not in sys.path:
            sys.path.append(p)


_ensure_paths()

import ml_dtypes  # noqa: E402


def _install_ntff_hook():
    """Register the NTFF profile hook (used when tracing; harmless otherwise)."""
    try:
        import antenv
    except ImportError:
        return
    if "antenv.axon_hooks" not in sys.modules:
        hooks_mod = types.ModuleType("antenv.axon_hooks")
        _hook = [None]
        hooks_mod.set_axon_ntff_profile_hook = lambda h: _hook.__setitem__(0, h)
        hooks_mod.get_axon_ntff_profile_hook = lambda: _hook[0]
        sys.modules["antenv.axon_hooks"] = hooks_mod
        antenv.axon_hooks = hooks_mod
    from antenv.axon_hooks import (
        get_axon_ntff_profile_hook,
        set_axon_ntff_profile_hook,
    )

    if get_axon_ntff_profile_hook() is None:
        try:
            from trn_agent_boot.trn_boot import _ntff_profile_via_ctypes

            so = "/opt/axon/libaxon_pjrt.so"
            if os.path.exists(so):
                set_axon_ntff_profile_hook(_ntff_profile_via_ctypes(so))
        except Exception:
            pass


_install_ntff_hook()

import concourse.bass as bass  # noqa: E402
import concourse.tile as tile  # noqa: E402
from concourse import bacc, mybir  # noqa: E402
import concourse.bass_utils as _bu  # noqa: E402

# Artifact upload needs a bucket that isn't reachable here; keep traces local.
_bu.upload_artifacts = lambda tmpdir: f"local:{tmpdir}"

BF16 = mybir.dt.bfloat16
F32 = mybir.dt.float32
NP_BF16 = ml_dtypes.bfloat16

C = 96          # channels (in == out)
B = 8           # num basis
K = 3           # kernel size
N_CORES = 8
H = W = 512     # input spatial
HO = WO = 256   # output spatial
HP = H + 2      # padded rows
WP = W + 2      # padded cols (phase-split: [257 even | 257 odd])
SH_ROWS = 257   # padded rows per shard (256 + 1 halo)
CORE_HO = 128   # output rows per core
BH = 16         # output rows per block
NBLK = CORE_HO // BH
IN_ROWS = 2 * BH + 1  # input rows per block (33)

# column base per kw tap: even-phase col 2*ow -> slot ow (base 0);
# odd-phase col 2*ow+1 -> slot 257+ow; even col 2*ow+2 -> slot ow+1.
_KW_BASE = {0: 0, 1: 257, 2: 1}

_PROGRAM_CACHE = {}


def _build_program():
    """One SPMD Bass program, shared by all 8 cores."""
    nc = bacc.Bacc()
    x_d = nc.dram_tensor("x", [C, SH_ROWS, WP], BF16, kind="ExternalInput")
    w_d = nc.dram_tensor("wt", [C, K * K, C], BF16, kind="ExternalInput")
    b_d = nc.dram_tensor("bias", [C, 1], F32, kind="ExternalInput")
    y_d = nc.dram_tensor("out", [C, CORE_HO, WO], F32, kind="ExternalOutput")

    with tile.TileContext(nc) as tc:
        with (
            tc.tile_pool(name="const", bufs=1) as cpool,
            tc.tile_pool(name="xin", bufs=3) as xpool,
            tc.tile_pool(name="oout", bufs=2) as opool,
            tc.tile_pool(name="psum", bufs=8, space=bass.MemorySpace.PSUM) as ppool,
        ):
            wt = cpool.tile([C, K * K, C], BF16)
            nc.sync.dma_start(wt[:], w_d[:])
            bias = cpool.tile([C, 1], F32)
            nc.sync.dma_start(bias[:], b_d[:])

            for blk in range(NBLK):
                xt = xpool.tile([C, IN_ROWS, WP], BF16)
                r0 = 2 * BH * blk
                # split the block load so matmuls can start on the first half
                # while the second half is still in flight
                half = IN_ROWS // 2 + 1  # 17
                nc.sync.dma_start(
                    xt[:, :half, :], x_d[:, r0 : r0 + half, :]
                )
                nc.sync.dma_start(
                    xt[:, half:, :], x_d[:, r0 + half : r0 + IN_ROWS, :]
                )
                out_sb = opool.tile([C, BH, WO], F32)
                for t in range(BH // 2):
                    ps = ppool.tile([C, 2 * WO], F32)
                    for tap in range(K * K):
                        kh, kw = tap // K, tap % K
                        cb = _KW_BASE[kw]
                        rhs = xt[:, 4 * t + kh : 4 * t + kh + 3 : 2, cb : cb + WO]
                        nc.tensor.matmul(
                            ps[:],
                            wt[:, tap, :],
                            rhs,
                            start=(tap == 0),
                            stop=(tap == K * K - 1),
                        )
                    nc.scalar.activation(
                        out_sb[:, 2 * t : 2 * t + 2, :],
                        ps[:].rearrange("p (a b) -> p a b", a=2),
                        mybir.ActivationFunctionType.Identity,
                        bias=bias[:],
                    )
                # output on the ACT HWDGE ring to decouple from input loads
                nc.scalar.dma_start(
                    y_d[:, BH * blk : BH * blk + BH, :], out_sb[:]
                )

    nc.compile()
    return nc


def _get_program():
    if "nc" not in _PROGRAM_CACHE:
        _PROGRAM_CACHE["nc"] = _build_program()
    return _PROGRAM_CACHE["nc"]


def _prepare_inputs(x, coeff, basis, bias):
    """Host prep: effective weights, padded phase-split bf16 x, shards."""
    # wt[i, tap, o] = sum_b coeff[o,i,b] * basis[b, tap]
    w_eff = (
        coeff.astype(np.float32).reshape(C * C, B)
        @ basis.astype(np.float32).reshape(B, K * K)
    ).reshape(C, C, K * K)
    wt = np.ascontiguousarray(w_eff.transpose(1, 2, 0)).astype(NP_BF16)

    xb = x.astype(NP_BF16)
    xph = np.zeros((x.shape[0], C, HP, WP), dtype=NP_BF16)
    # even phase: padded col 2j -> orig col 2j-1  (slot j=1..256)
    xph[:, :, 1 : H + 1, 1:257] = xb[:, :, :, 1::2]
    # odd phase: padded col 2j+1 -> orig col 2j  (slot 257+j, j=0..255)
    xph[:, :, 1 : H + 1, 257:513] = xb[:, :, :, 0::2]

    bias2 = np.ascontiguousarray(bias.astype(np.float32).reshape(C, 1))

    in_maps = []
    for s in range(N_CORES):
        b_idx, h_idx = divmod(s, 2)
        shard = np.ascontiguousarray(
            xph[b_idx, :, 256 * h_idx : 256 * h_idx + SH_ROWS, :]
        )
        in_maps.append({"x": shard, "wt": wt, "bias": bias2})
    return in_maps


def _assemble(results, n_batch):
    out = np.empty((n_batch, C, 2 * CORE_HO, WO), dtype=np.float32)
    for s in range(N_CORES):
        b_idx, h_idx = divmod(s, 2)
        out[b_idx, :, CORE_HO * h_idx : CORE_HO * (h_idx + 1), :] = results[s]["out"]
    return out


def run(x, coeff, basis, bias, trace=False, trace_cores=None):
    """Run the kernel; returns (full_output, BassKernelResults)."""
    nc = _get_program()
    in_maps = _prepare_inputs(x, coeff, basis, bias)
    res = _bu.run_bass_kernel_spmd(
        nc,
        in_maps,
        list(range(N_CORES)),
        trace=trace,
        trace_cores=trace_cores,
    )
    return _assemble(res.results, x.shape[0]), res


def kernel(x, coeff, basis, bias):
    out, _ = run(x, coeff, basis, bias, trace=False)
    return out


# revision 8
# speedup vs baseline: 1.2156x; 1.0708x over previous
"""DISCO downsample conv (3x3, stride 2, pad 1) on 8 Trainium2 NeuronCores.

Strategy:
  - Effective weights w[o,i,kh,kw] = sum_b coeff[o,i,b]*basis[b,kh,kw] are tiny:
    computed on host, shipped per-tap transposed as wt[i, tap, o] (bf16).
  - x is zero-padded (H+2, W+2) on host, W phase-split into [even | odd]
    columns so every conv tap reads a contiguous run of 256 columns, and cast
    to bf16 (halves HBM traffic; PSUM accumulation stays fp32).
  - Sharding: 8 shards = (batch b in 0..3) x (H half in 0..1). Each core gets
    padded rows [256*h, 256*h + 257) of batch b -- the 1-row halo is part of
    the shard, so no inter-core communication is needed.
  - Per core: stream 8 row-blocks (16 output rows each = 33 input rows).
    For each pair of output rows: one PSUM bank [96, 512], accumulate 9
    matmuls (K=96 in-channels, M=96 out-channels, N=512 pixels), then
    ScalarE adds bias while copying PSUM -> SBUF, and the block is DMA'd out.
"""

import os
import sys
import types

import numpy as np


# ----------------------------------------------------------------------------
# Environment bootstrap (self-contained: no reads from /root/problem).
# ----------------------------------------------------------------------------
def _ensure_paths():
    for p in (
        "/root/.axon_site",
        "/root/.axon_site/_ro/trn_rl_repo",
        "/root/.axon_site/_ro/pypackages",
        "/opt/trn_rl_repo",
    ):
        if os.path.isdir(p) and p not in sys.path:
            sys.path.append(p)


_ensure_paths()

import ml_dtypes  # noqa: E402


def _install_ntff_hook():
    """Register the NTFF profile hook (used when tracing; harmless otherwise)."""
    try:
        import antenv
    except ImportError:
        return
    if "antenv.axon_hooks" not in sys.modules:
        hooks_mod = types.ModuleType("antenv.axon_hooks")
        _hook = [None]
        hooks_mod.set_axon_ntff_profile_hook = lambda h: _hook.__setitem__(0, h)
        hooks_mod.get_axon_ntff_profile_hook = lambda: _hook[0]
        sys.modules["antenv.axon_hooks"] = hooks_mod
        antenv.axon_hooks = hooks_mod
    from antenv.axon_hooks import (
        get_axon_ntff_profile_hook,
        set_axon_ntff_profile_hook,
    )

    if get_axon_ntff_profile_hook() is None:
        try:
            from trn_agent_boot.trn_boot import _ntff_profile_via_ctypes

            so = "/opt/axon/libaxon_pjrt.so"
            if os.path.exists(so):
                set_axon_ntff_profile_hook(_ntff_profile_via_ctypes(so))
        except Exception:
            pass


_install_ntff_hook()

import concourse.bass as bass  # noqa: E402
import concourse.tile as tile  # noqa: E402
from concourse import bacc, mybir  # noqa: E402
import concourse.bass_utils as _bu  # noqa: E402

# Artifact upload needs a bucket that isn't reachable here; keep traces local.
_bu.upload_artifacts = lambda tmpdir: f"local:{tmpdir}"

BF16 = mybir.dt.bfloat16
F32 = mybir.dt.float32
NP_BF16 = ml_dtypes.bfloat16

C = 96          # channels (in == out)
B = 8           # num basis
K = 3           # kernel size
N_CORES = 8
H = W = 512     # input spatial
HO = WO = 256   # output spatial
HP = H + 2      # padded rows
WP = W + 2      # padded cols (phase-split: [257 even | 257 odd])
SH_ROWS = 257   # padded rows per shard (256 + 1 halo)
CORE_HO = 128   # output rows per core
BH = 32         # output rows per block
NBLK = CORE_HO // BH
IN_ROWS = 2 * BH + 1  # input rows per block (65)
# input-chunk row splits within a block (finer at the front so the PE can
# start as soon as the first rows land)
IN_CHUNKS = (9, 8, 16, 16, 16)
# output stored as bf16 to halve write traffic (accumulation stays fp32)
OUT_BF16 = os.environ.get("KERNEL_OUT_F32", "") != "1"

# column base per kw tap: even-phase col 2*ow -> slot ow (base 0);
# odd-phase col 2*ow+1 -> slot 257+ow; even col 2*ow+2 -> slot ow+1.
_KW_BASE = {0: 0, 1: 257, 2: 1}

_PROGRAM_CACHE = {}


def _build_program():
    """One SPMD Bass program, shared by all 8 cores."""
    nc = bacc.Bacc()
    out_dt = BF16 if OUT_BF16 else F32
    x_d = nc.dram_tensor("x", [C, SH_ROWS, WP], BF16, kind="ExternalInput")
    w_d = nc.dram_tensor("wt", [C, K * K, C], BF16, kind="ExternalInput")
    b_d = nc.dram_tensor("bias", [C, 1], F32, kind="ExternalInput")
    y_d = nc.dram_tensor("out", [C, CORE_HO, WO], out_dt, kind="ExternalOutput")

    with tile.TileContext(nc) as tc:
        with (
            tc.tile_pool(name="const", bufs=1) as cpool,
            tc.tile_pool(name="xin", bufs=2) as xpool,
            tc.tile_pool(name="oout", bufs=2) as opool,
            tc.tile_pool(name="psum", bufs=8, space=bass.MemorySpace.PSUM) as ppool,
        ):
            wt = cpool.tile([C, K * K, C], BF16)
            nc.sync.dma_start(wt[:], w_d[:])
            bias = cpool.tile([C, 1], F32)
            nc.sync.dma_start(bias[:], b_d[:])

            for blk in range(NBLK):
                xt = xpool.tile([C, IN_ROWS, WP], BF16)
                r0 = 2 * BH * blk
                # chunked block load: matmuls start on early rows while the
                # rest of the block is still in flight
                rr = 0
                for nrows in IN_CHUNKS:
                    nc.sync.dma_start(
                        xt[:, rr : rr + nrows, :],
                        x_d[:, r0 + rr : r0 + rr + nrows, :],
                    )
                    rr += nrows
                assert rr == IN_ROWS
                out_sb = opool.tile([C, BH, WO], out_dt)
                for t in range(BH // 2):
                    ps = ppool.tile([C, 2 * WO], F32)
                    for tap in range(K * K):
                        kh, kw = tap // K, tap % K
                        cb = _KW_BASE[kw]
                        rhs = xt[:, 4 * t + kh : 4 * t + kh + 3 : 2, cb : cb + WO]
                        nc.tensor.matmul(
                            ps[:],
                            wt[:, tap, :],
                            rhs,
                            start=(tap == 0),
                            stop=(tap == K * K - 1),
                        )
                    nc.scalar.activation(
                        out_sb[:, 2 * t : 2 * t + 2, :],
                        ps[:].rearrange("p (a b) -> p a b", a=2),
                        mybir.ActivationFunctionType.Identity,
                        bias=bias[:],
                    )
                    # flush each half-block on the ACT HWDGE ring as soon as
                    # it is complete, decoupled from the input ring
                    if t == BH // 4 - 1:
                        nc.scalar.dma_start(
                            y_d[:, BH * blk : BH * blk + BH // 2, :],
                            out_sb[:, : BH // 2, :],
                        )
                    elif t == BH // 2 - 1:
                        nc.scalar.dma_start(
                            y_d[:, BH * blk + BH // 2 : BH * blk + BH, :],
                            out_sb[:, BH // 2 :, :],
                        )

    nc.compile()
    return nc


def _get_program():
    if "nc" not in _PROGRAM_CACHE:
        _PROGRAM_CACHE["nc"] = _build_program()
    return _PROGRAM_CACHE["nc"]


def _prepare_inputs(x, coeff, basis, bias):
    """Host prep: effective weights, padded phase-split bf16 x, shards."""
    # wt[i, tap, o] = sum_b coeff[o,i,b] * basis[b, tap]
    w_eff = (
        coeff.astype(np.float32).reshape(C * C, B)
        @ basis.astype(np.float32).reshape(B, K * K)
    ).reshape(C, C, K * K)
    wt = np.ascontiguousarray(w_eff.transpose(1, 2, 0)).astype(NP_BF16)

    xb = x.astype(NP_BF16)
    xph = np.zeros((x.shape[0], C, HP, WP), dtype=NP_BF16)
    # even phase: padded col 2j -> orig col 2j-1  (slot j=1..256)
    xph[:, :, 1 : H + 1, 1:257] = xb[:, :, :, 1::2]
    # odd phase: padded col 2j+1 -> orig col 2j  (slot 257+j, j=0..255)
    xph[:, :, 1 : H + 1, 257:513] = xb[:, :, :, 0::2]

    bias2 = np.ascontiguousarray(bias.astype(np.float32).reshape(C, 1))

    in_maps = []
    for s in range(N_CORES):
        b_idx, h_idx = divmod(s, 2)
        shard = np.ascontiguousarray(
            xph[b_idx, :, 256 * h_idx : 256 * h_idx + SH_ROWS, :]
        )
        in_maps.append({"x": shard, "wt": wt, "bias": bias2})
    return in_maps


def _assemble(results, n_batch):
    out = np.empty((n_batch, C, 2 * CORE_HO, WO), dtype=np.float32)
    for s in range(N_CORES):
        b_idx, h_idx = divmod(s, 2)
        out[b_idx, :, CORE_HO * h_idx : CORE_HO * (h_idx + 1), :] = results[s][
            "out"
        ].astype(np.float32)
    return out


def run(x, coeff, basis, bias, trace=False, trace_cores=None):
    """Run the kernel; returns (full_output, BassKernelResults)."""
    nc = _get_program()
    in_maps = _prepare_inputs(x, coeff, basis, bias)
    res = _bu.run_bass_kernel_spmd(
        nc,
        in_maps,
        list(range(N_CORES)),
        trace=trace,
        trace_cores=trace_cores,
    )
    return _assemble(res.results, x.shape[0]), res


def kernel(x, coeff, basis, bias):
    out, _ = run(x, coeff, basis, bias, trace=False)
    return out


# revision 11
# speedup vs baseline: 1.2402x; 1.0202x over previous
"""DISCO downsample conv (3x3, stride 2, pad 1) on 8 Trainium2 NeuronCores.

Strategy:
  - Effective weights w[o,i,kh,kw] = sum_b coeff[o,i,b]*basis[b,kh,kw] are tiny:
    computed on host, shipped per-tap transposed as wt[i, tap, o] (bf16).
  - x is zero-padded (H+2, W+2) on host, W phase-split into [even | odd]
    columns so every conv tap reads a contiguous run of 256 columns, and cast
    to bf16 (halves HBM traffic; PSUM accumulation stays fp32).
  - Sharding: 8 shards = (batch b in 0..3) x (H half in 0..1). Each core gets
    padded rows [256*h, 256*h + 257) of batch b -- the 1-row halo is part of
    the shard, so no inter-core communication is needed.
  - Per core: stream 8 row-blocks (16 output rows each = 33 input rows).
    For each pair of output rows: one PSUM bank [96, 512], accumulate 9
    matmuls (K=96 in-channels, M=96 out-channels, N=512 pixels), then
    ScalarE adds bias while copying PSUM -> SBUF, and the block is DMA'd out.
"""

import os
import sys
import types

import numpy as np


# ----------------------------------------------------------------------------
# Environment bootstrap (self-contained: no reads from /root/problem).
# ----------------------------------------------------------------------------
def _ensure_paths():
    for p in (
        "/root/.axon_site",
        "/root/.axon_site/_ro/trn_rl_repo",
        "/root/.axon_site/_ro/pypackages",
        "/opt/trn_rl_repo",
    ):
        if os.path.isdir(p) and p not in sys.path:
            sys.path.append(p)


_ensure_paths()

import ml_dtypes  # noqa: E402


def _install_ntff_hook():
    """Register the NTFF profile hook (used when tracing; harmless otherwise)."""
    try:
        import antenv
    except ImportError:
        return
    if "antenv.axon_hooks" not in sys.modules:
        hooks_mod = types.ModuleType("antenv.axon_hooks")
        _hook = [None]
        hooks_mod.set_axon_ntff_profile_hook = lambda h: _hook.__setitem__(0, h)
        hooks_mod.get_axon_ntff_profile_hook = lambda: _hook[0]
        sys.modules["antenv.axon_hooks"] = hooks_mod
        antenv.axon_hooks = hooks_mod
    from antenv.axon_hooks import (
        get_axon_ntff_profile_hook,
        set_axon_ntff_profile_hook,
    )

    if get_axon_ntff_profile_hook() is None:
        try:
            from trn_agent_boot.trn_boot import _ntff_profile_via_ctypes

            so = "/opt/axon/libaxon_pjrt.so"
            if os.path.exists(so):
                set_axon_ntff_profile_hook(_ntff_profile_via_ctypes(so))
        except Exception:
            pass


_install_ntff_hook()

import concourse.bass as bass  # noqa: E402
import concourse.tile as tile  # noqa: E402
from concourse import bacc, mybir  # noqa: E402
import concourse.bass_utils as _bu  # noqa: E402

# Artifact upload needs a bucket that isn't reachable here; keep traces local.
_bu.upload_artifacts = lambda tmpdir: f"local:{tmpdir}"

BF16 = mybir.dt.bfloat16
F32 = mybir.dt.float32
NP_BF16 = ml_dtypes.bfloat16

C = 96          # channels (in == out)
B = 8           # num basis
K = 3           # kernel size
N_CORES = 8
H = W = 512     # input spatial
HO = WO = 256   # output spatial
HP = H + 2      # padded rows
WP = W + 2      # padded cols (phase-split: [257 even | 257 odd])
SH_ROWS = 257   # padded rows per shard (256 + 1 halo)
CORE_HO = 128   # output rows per core
BH = 32         # output rows per block
NBLK = CORE_HO // BH
IN_ROWS = 2 * BH + 1  # input rows per block (65)
# input-chunk row splits within a block (finer at the front so the PE can
# start as soon as the first rows land)
IN_CHUNKS = (9, 8, 16, 16, 16)
# output stored as bf16 to halve write traffic (accumulation stays fp32)
OUT_BF16 = os.environ.get("KERNEL_OUT_F32", "") != "1"

# column base per kw tap: even-phase col 2*ow -> slot ow (base 0);
# odd-phase col 2*ow+1 -> slot 257+ow; even col 2*ow+2 -> slot ow+1.
_KW_BASE = {0: 0, 1: 257, 2: 1}

_PROGRAM_CACHE = {}


def _build_program():
    """One SPMD Bass program, shared by all 8 cores."""
    nc = bacc.Bacc()
    out_dt = BF16 if OUT_BF16 else F32
    x_d = nc.dram_tensor("x", [C, SH_ROWS, WP], BF16, kind="ExternalInput")
    w_d = nc.dram_tensor("wt", [C, K * K, C], BF16, kind="ExternalInput")
    b_d = nc.dram_tensor("bias", [C, 1], F32, kind="ExternalInput")
    y_d = nc.dram_tensor("out", [C, CORE_HO, WO], out_dt, kind="ExternalOutput")

    with tile.TileContext(nc) as tc:
        with (
            tc.tile_pool(name="const", bufs=1) as cpool,
            tc.tile_pool(name="xin", bufs=2) as xpool,
            tc.tile_pool(name="oout", bufs=2) as opool,
            tc.tile_pool(name="psum", bufs=8, space=bass.MemorySpace.PSUM) as ppool,
        ):
            # constants ride the ACT ring so the first x chunk is not queued
            # behind them on the input ring
            wt = cpool.tile([C, K * K, C], BF16)
            nc.scalar.dma_start(wt[:], w_d[:])
            bias = cpool.tile([C, 1], F32)
            nc.scalar.dma_start(bias[:], b_d[:])

            for blk in range(NBLK):
                xt = xpool.tile([C, IN_ROWS, WP], BF16)
                r0 = 2 * BH * blk
                # chunked block load: matmuls start on early rows while the
                # rest of the block is still in flight
                rr = 0
                chunks = (5, 4, 8, 16, 16, 16) if blk == 0 else IN_CHUNKS
                for nrows in chunks:
                    nc.sync.dma_start(
                        xt[:, rr : rr + nrows, :],
                        x_d[:, r0 + rr : r0 + rr + nrows, :],
                    )
                    rr += nrows
                assert rr == IN_ROWS
                out_sb = opool.tile([C, BH, WO], out_dt)
                for t in range(BH // 2):
                    ps = ppool.tile([C, 2 * WO], F32)
                    for tap in range(K * K):
                        kh, kw = tap // K, tap % K
                        cb = _KW_BASE[kw]
                        rhs = xt[:, 4 * t + kh : 4 * t + kh + 3 : 2, cb : cb + WO]
                        nc.tensor.matmul(
                            ps[:],
                            wt[:, tap, :],
                            rhs,
                            start=(tap == 0),
                            stop=(tap == K * K - 1),
                        )
                    nc.scalar.activation(
                        out_sb[:, 2 * t : 2 * t + 2, :],
                        ps[:].rearrange("p (a b) -> p a b", a=2),
                        mybir.ActivationFunctionType.Identity,
                        bias=bias[:],
                    )
                    # flush finished output rows on the ACT HWDGE ring as soon
                    # as they are complete, decoupled from the input ring; the
                    # last block flushes in quarters to shrink the kernel tail
                    flush_at = (
                        (BH // 4 - 1, BH // 2 - 1)
                        if blk < NBLK - 1
                        else (BH // 8 - 1, BH // 4 - 1, 3 * BH // 8 - 1, BH // 2 - 1)
                    )
                    if t in flush_at:
                        prev = 0 if t == flush_at[0] else (flush_at[flush_at.index(t) - 1] + 1)
                        lo, hi = 2 * prev, 2 * t + 2
                        nc.scalar.dma_start(
                            y_d[:, BH * blk + lo : BH * blk + hi, :],
                            out_sb[:, lo:hi, :],
                        )

    nc.compile()
    return nc


def _get_program():
    if "nc" not in _PROGRAM_CACHE:
        _PROGRAM_CACHE["nc"] = _build_program()
    return _PROGRAM_CACHE["nc"]


def _prepare_inputs(x, coeff, basis, bias):
    """Host prep: effective weights, padded phase-split bf16 x, shards."""
    # wt[i, tap, o] = sum_b coeff[o,i,b] * basis[b, tap]
    w_eff = (
        coeff.astype(np.float32).reshape(C * C, B)
        @ basis.astype(np.float32).reshape(B, K * K)
    ).reshape(C, C, K * K)
    wt = np.ascontiguousarray(w_eff.transpose(1, 2, 0)).astype(NP_BF16)

    xb = x.astype(NP_BF16)
    xph = np.zeros((x.shape[0], C, HP, WP), dtype=NP_BF16)
    # even phase: padded col 2j -> orig col 2j-1  (slot j=1..256)
    xph[:, :, 1 : H + 1, 1:257] = xb[:, :, :, 1::2]
    # odd phase: padded col 2j+1 -> orig col 2j  (slot 257+j, j=0..255)
    xph[:, :, 1 : H + 1, 257:513] = xb[:, :, :, 0::2]

    bias2 = np.ascontiguousarray(bias.astype(np.float32).reshape(C, 1))

    in_maps = []
    for s in range(N_CORES):
        b_idx, h_idx = divmod(s, 2)
        shard = np.ascontiguousarray(
            xph[b_idx, :, 256 * h_idx : 256 * h_idx + SH_ROWS, :]
        )
        in_maps.append({"x": shard, "wt": wt, "bias": bias2})
    return in_maps


def _assemble(results, n_batch):
    out = np.empty((n_batch, C, 2 * CORE_HO, WO), dtype=np.float32)
    for s in range(N_CORES):
        b_idx, h_idx = divmod(s, 2)
        out[b_idx, :, CORE_HO * h_idx : CORE_HO * (h_idx + 1), :] = results[s][
            "out"
        ].astype(np.float32)
    return out


def run(x, coeff, basis, bias, trace=False, trace_cores=None):
    """Run the kernel; returns (full_output, BassKernelResults)."""
    nc = _get_program()
    in_maps = _prepare_inputs(x, coeff, basis, bias)
    res = _bu.run_bass_kernel_spmd(
        nc,
        in_maps,
        list(range(N_CORES)),
        trace=trace,
        trace_cores=trace_cores,
    )
    return _assemble(res.results, x.shape[0]), res


def kernel(x, coeff, basis, bias):
    out, _ = run(x, coeff, basis, bias, trace=False)
    return out
